# revision 36
# baseline (speedup 1.0000x reference)
"""CFConv Trainium2 kernel.

Math: out[b,o,y,x] = sum_{k,i,j} weight[k,o,i,j] * fa[b,i,y+dy,x+dx] * wa[b,j,y+dy,x+dx]
(3x3 valid conv over the outer-product channel space of fa (65ch) x wa (17ch)).

Strategy (8 NeuronCores, SPMD):
- Shard (batch b, row-half h): each core computes 63 output rows of one batch.
- On-chip, form z[(i,j), pix] = f_i * w_j for the 64x16 "main" (i,j) grid as
  8 partition-chunks of 128 (j-minor within 16-partition groups). The
  replicated-f factor comes from stream_shuffle (chunks 0-3) and from
  host-prepped windows DMA'd per pair (chunks 4-7) — the split keeps both
  the vector engine (~73% busy) and the shared DMA fabric (~150 GB/s/core
  across 8 cores) under their budgets. z = frep * wt on DVE (fp16 2x).
  The remaining 81 channels (ones-augmented) are read directly from a
  packed [f; w; ones] tensor (chunk "8").
- Contract on the tensor engine in fp16 (fp32 PSUM). Matmuls are issued in
  column-tiled pairs (tile_position (0,0)/(0,64)): two 64-wide PE column
  groups concurrently compute two adjacent pixel tiles into the two
  partition halves of one PSUM bank; pair-slot = N cycles at 2.4 GHz.
- Head choreography: DMA rings are in-order and share one DMA engine +
  the 8-core fabric, so the head runs on two rings (sync+scalar) with
  need-ordered small windows; the gpsimd ring is gated behind a vector
  WAW-memset so it cannot steal head bandwidth. Warmup matmuls (memset-fed)
  hold the PE clock (HAM) at 2.4 GHz until data lands; pair 1's chunk-8
  matmuls are hoisted before pair 0's main chunks to double the PE runway
  while the first z chunks are built.
- Pair 7 covers the tail 7 rows as two N=448 groups (no duplicated row).
- Output layout stays at input width (128): all 9 conv offsets are plain
  column shifts; garbage columns are skipped at DMA-out. PSUM->SBUF
  staging runs on the scalar (Act) engine.
"""

import numpy as np

B, WCH, FCH, OCH, H, W = 4, 16, 64, 64, 128, 128
KX = 3
HO = WO = H - KX + 1          # 126
ROWS_OUT = 63                 # output rows per core
ROWS_IN = 65                  # input rows per core
FREE = 8448                   # padded region width (66 rows * 128)
VALID = ROWS_IN * W           # 8320
NPAIR = 8                     # pixel-tile pairs per core
HALO = 2 * W + 2              # 258
WIN = 1024 + HALO             # 1282: z window per full pair

_cache = {}


def _pair_c0(a):
    return 1024 * a if a < NPAIR - 1 else 7168   # pair 7: rows 56-62, N=448


def _pair_n(a):
    return 512 if a < NPAIR - 1 else 448


def _build_program():
    import concourse.bacc as bacc
    import concourse.mybir as mybir
    import concourse.tile as tile

    f16 = mybir.dt.float16
    f32 = mybir.dt.float32

    nc = bacc.Bacc("TRN2", target_bir_lowering=False)
    fw_d = nc.dram_tensor("fw", (81, FREE), f16, kind="ExternalInput")
    fpre_d = nc.dram_tensor("fpre", (128, FREE), f16, kind="ExternalInput")
    fpw_d = nc.dram_tensor("fpw", (128, NPAIR * 3 * WIN), f16, kind="ExternalInput")
    # extra channels for offsets k=0..5, packed 6x81=486 rows -> 4 full
    # 128-row tiles with the column shift baked in host-side (saves two
    # PE pair-slots per pair vs running those offsets at 81/128 rows).
    fwp2_d = nc.dram_tensor("fwp2", (128, 4 * FREE), f16, kind="ExternalInput")
    wkp2_d = nc.dram_tensor("wkp2", (128, 4 * 64), f16, kind="ExternalInput")
    wt_d = nc.dram_tensor("wt", (128, FREE), f16, kind="ExternalInput")
    wkm_d = nc.dram_tensor("wkm", (128, 9 * 8 * 64), f16, kind="ExternalInput")
    wkx_d = nc.dram_tensor("wkx", (81, 9 * 64), f16, kind="ExternalInput")
    # output stays at input width (128) and fp16: every out-DMA is then a
    # contiguous 64-descriptor transfer; host slices off the 2 pad columns.
    out_d = nc.dram_tensor("out", (OCH, ROWS_OUT * W), f16, kind="ExternalOutput")

    with tile.TileContext(nc) as tc:
        with tc.tile_pool(name="inp", bufs=1) as inp, \
             tc.tile_pool(name="frep", bufs=2) as frp, \
             tc.tile_pool(name="z", bufs=2) as zp, \
             tc.tile_pool(name="st", bufs=3) as stp, \
             tc.tile_pool(name="ps", bufs=4, space="PSUM") as psp:
            # memset-fed warmup matmuls hold the PE clock warm until the
            # first fw window lands (~16us); their PSUM is never read.
            warm = inp.tile([128, 256], f16)
            nc.gpsimd.memset(warm[:], 0.0)
            warm_ps = psp.tile([128, 512], f32, bufs=1)
            for _ in range(58):
                nc.tensor.matmul(warm_ps[0:64, 0:256], warm[:, 0:64], warm[:, 0:256],
                                 start=True, stop=True, tile_position=(0, 0))

            fw_s = inp.tile([81, FREE], f16)
            fpre_s = inp.tile([128, FREE], f16)
            wt_s = inp.tile([128, FREE], f16)
            wkm_s = inp.tile([128, 9 * 8 * 64], f16)
            wkx_s = inp.tile([81, 9 * 64], f16)
            wkp2_s = inp.tile([128, 4 * 64], f16)

            # --- head: two rings, z-chain first, PE inputs after -------
            # (the PE waits warm behind the warmups; the z chain cannot)
            nc.sync.dma_start(fpre_s[:, 0:672], fpre_d[:, 0:672])
            nc.scalar.dma_start(wkx_s[:], wkx_d[:])
            nc.scalar.dma_start(fpre_s[:, 672:1344], fpre_d[:, 672:1344])
            nc.sync.dma_start(wt_s[:, 0:672], wt_d[:, 0:672])
            nc.scalar.dma_start(wt_s[:, 672:1344], wt_d[:, 672:1344])
            nc.sync.dma_start(fw_s[:, 0:1284], fw_d[:, 0:1284])
            nc.scalar.dma_start(fw_s[:, 1284:2368], fw_d[:, 1284:2368])
            nc.sync.dma_start(wkm_s[:, 0:1152], wkm_d[:, 0:1152])
            nc.sync.dma_start(wkp2_s[:], wkp2_d[:])
            nc.scalar.dma_start(wkm_s[:, 1152:], wkm_d[:, 1152:])
            # bulk width, woven into the sync ring in need-order
            sl0 = slice(1344, 3712)
            sl1 = slice(3712, 6080)
            sl2 = slice(6080, FREE)
            fb0 = slice(2368, 4736)
            fb1 = slice(4736, FREE)
            bulk = {
                0: [(fpre_s, fpre_d, sl0), (wt_s, wt_d, sl0)],
                1: [(fw_s, fw_d, fb0), (fpre_s, fpre_d, sl1), (wt_s, wt_d, sl1)],
                2: [(fw_s, fw_d, fb1)],
                3: [(fpre_s, fpre_d, sl2), (wt_s, wt_d, sl2)],
            }

            for a in range(NPAIR):
                c0, n = _pair_c0(a), _pair_n(a)
                win = min(WIN, FREE - c0)
                zz = []
                for c in range(8):
                    fr = frp.tile([128, WIN], f16, tag=f"fc{c}")
                    if c < 5:
                        mask = [2 * c + (r // 16) for r in range(32)]
                        nc.vector.stream_shuffle(fr[:, 0:win], fpre_s[:, c0:c0 + win], mask)
                    else:
                        src = fpw_d[:, (a * 3 + c - 5) * WIN:(a * 3 + c - 4) * WIN]
                        eng = nc.scalar if c == 6 else nc.sync
                        eng.dma_start(fr[:], src)
                    z = zp.tile([128, WIN], f16, tag=f"z{c}")
                    nc.vector.tensor_mul(z[:, 0:win], fr[:, 0:win], wt_s[:, c0:c0 + win])
                    zz.append(z)
                for dst, srcd, sl in bulk.get(a, []):
                    nc.sync.dma_start(dst[:, sl], srcd[:, sl])

                # baked-shift packed extra channels for offsets k=0..2
                fps = []
                for t in (0, 1, 2, 3):
                    fp = frp.tile([128, 2 * 512], f16, tag=f"fp{t}")
                    eng = nc.scalar if t % 2 == 0 else nc.sync
                    eng.dma_start(fp[:, 0:2 * n],
                                  fwp2_d[:, t * FREE + c0:t * FREE + c0 + 2 * n])
                    fps.append(fp)

                ps = psp.tile([128, 512], f32, tag="ps")
                for c in (8, 0, 1, 2, 3, 4, 5, 6, 7):
                    for k in (range(6, 9) if c == 8 else range(9)):
                        dy, dx = divmod(k, KX)
                        d = dy * W + dx
                        for g, off in ((0, 0), (1, n)):
                            if c < 8:
                                lhsT = wkm_s[:, (c * 9 + k) * 64:(c * 9 + k) * 64 + 64]
                                rhs = zz[c][:, d + off:d + off + n]
                            else:
                                lhsT = wkx_s[:, k * 64:k * 64 + 64]
                                rhs = fw_s[:, c0 + d + off:c0 + d + off + n]
                            nc.tensor.matmul(
                                ps[64 * g:64 * g + 64, 0:n], lhsT, rhs,
                                start=(c == 8 and k == 6),
                                stop=False,
                                tile_position=(0, 64 * g),
                            )
                # packed extra slots last: their windows arrive mid-pair
                for t in (0, 1, 2, 3):
                    for g, off in ((0, 0), (1, n)):
                        nc.tensor.matmul(
                            ps[64 * g:64 * g + 64, 0:n],
                            wkp2_s[:, t * 64:t * 64 + 64],
                            fps[t][:, off:off + n],
                            start=False, stop=(t == 3),
                            tile_position=(0, 64 * g),
                        )

                stage = stp.tile([128, 512], f16)
                nc.scalar.copy(stage[:, 0:n], ps[:, 0:n])
                # tail pair's output splits across both rings
                eng1 = nc.sync if a == NPAIR - 1 else nc.scalar
                nc.scalar.dma_start(out_d[:, c0:c0 + n], stage[0:64, 0:n])
                eng1.dma_start(out_d[:, c0 + n:c0 + 2 * n], stage[64:128, 0:n])

    nc.finalize()
    return nc


def _prep_core(inputf, inputw, b, h):
    r0 = 63 * h
    f_reg = np.zeros((64, FREE), np.float16)
    f_reg[:, :VALID] = inputf[b, :, r0:r0 + ROWS_IN, :].reshape(64, VALID)
    w_reg = np.zeros((16, FREE), np.float16)
    w_reg[:, :VALID] = inputw[b, :, r0:r0 + ROWS_IN, :].reshape(16, VALID)
    ones_reg = np.zeros((1, FREE), np.float16)
    ones_reg[0, :VALID] = 1.0
    fw = np.concatenate([f_reg, w_reg, ones_reg], 0)

    # pre-replicated f windows for the DMA-fed chunks 5-7, pair-major
    fpw = np.zeros((128, NPAIR * 3 * WIN), np.float16)
    for c in range(5, 8):
        frep = np.repeat(f_reg[8 * c:8 * c + 8], 16, axis=0)  # [128, FREE]
        for a in range(NPAIR):
            c0 = _pair_c0(a)
            win = min(WIN, FREE - c0)
            fpw[:, (a * 3 + c - 5) * WIN:(a * 3 + c - 5) * WIN + win] = \
                frep[:, c0:c0 + win]

    # shuffle-source layout for chunks 0-3 (quadrant-permuted f rows)
    fpre = np.zeros((128, FREE), np.float16)
    q = np.arange(4)[:, None]
    s = np.arange(16)[None, :]
    rows = (8 * (s // 2) + 2 * q + (s % 2)).reshape(-1)
    idx = (32 * q + s).reshape(-1)
    fpre[idx] = f_reg[rows]

    wt = np.empty((128, FREE), np.float16)
    for u in range(8):
        wt[16 * u:16 * u + 16] = w_reg

    # extra channels for offsets k=0..5 with the column shift baked in:
    # rows g = k*81+e packed into 4 tiles of 128
    fwp2 = np.zeros((4 * 128, FREE), np.float16)
    for k in range(6):
        d = (k // KX) * W + (k % KX)
        fwp2[k * 81:(k + 1) * 81, :FREE - d] = fw[:, d:]
    fwp2 = np.concatenate([fwp2[t * 128:(t + 1) * 128] for t in range(4)],
                          axis=1)  # [128, 4*FREE]
    return fw, fpre, fpw, wt, fwp2


def kernel(inputw, inputf, weight):
    from concourse import bass_utils

    inputw = np.asarray(inputw, np.float32)
    inputf = np.asarray(inputf, np.float32)
    weight = np.asarray(weight, np.float32)

    if "nc" not in _cache:
        _cache["nc"] = _build_program()
    nc = _cache["nc"]

    # weight layouts (replicated across cores)
    p = np.arange(128)
    wkm = np.empty((128, 8, 9, 64), np.float16)
    for t in range(8):
        iw = 8 * t + p // 16
        jw = p % 16
        wkm[:, t, :, :] = weight[:, :, iw, jw].transpose(2, 0, 1)
    wkm = wkm.reshape(128, 8 * 9 * 64)
    wkx = np.empty((81, 9, 64), np.float16)
    wkx[:64] = weight[:, :, :64, 16].transpose(2, 0, 1)
    wkx[64:80] = weight[:, :, 64, :16].transpose(2, 0, 1)
    wkx[80] = weight[:, :, 64, 16]
    wkx = wkx.reshape(81, 9 * 64)
    # packed-extra weights: row g = k*81+e of 2 tiles x 64 out channels
    wkp2 = np.zeros((4 * 128, 64), np.float16)
    for k in range(6):
        wkp2[k * 81:(k + 1) * 81] = wkx.reshape(81, 9, 64)[:, k, :]
    wkp2 = np.concatenate([wkp2[t * 128:(t + 1) * 128] for t in range(4)],
                          axis=1)  # [128, 4*64]

    in_maps = []
    for core in range(8):
        b, h = divmod(core, 2)
        fw, fpre, fpw, wt, fwp2 = _prep_core(inputf, inputw, b, h)
        in_maps.append({"fw": fw, "fpre": fpre, "fpw": fpw, "wt": wt,
                        "wkm": wkm, "wkx": wkx, "fwp2": fwp2, "wkp2": wkp2})

    res = bass_utils.run_bass_kernel_spmd(nc, in_maps, core_ids=list(range(8)))
    kernel.last_result = res

    out = np.empty((B, OCH, HO, WO), np.float32)
    for core in range(8):
        b, h = divmod(core, 2)
        full = res.results[core]["out"].reshape(OCH, ROWS_OUT, W)
        out[b, :, 63 * h:63 * h + 63, :] = full[:, :, 0:WO].astype(np.float32)
    return out


# revision 37
# speedup vs baseline: 1.0257x; 1.0257x over previous
"""CFConv Trainium2 kernel.

Math: out[b,o,y,x] = sum_{k,i,j} weight[k,o,i,j] * fa[b,i,y+dy,x+dx] * wa[b,j,y+dy,x+dx]
(3x3 valid conv over the outer-product channel space of fa (65ch) x wa (17ch)).

Strategy (8 NeuronCores, SPMD):
- Shard (batch b, row-half h): each core computes 63 output rows of one batch.
- On-chip, form z[(i,j), pix] = f_i * w_j for the 64x16 "main" (i,j) grid as
  8 partition-chunks of 128 (j-minor within 16-partition groups). The
  replicated-f factor comes from stream_shuffle (chunks 0-3) and from
  host-prepped windows DMA'd per pair (chunks 4-7) — the split keeps both
  the vector engine (~73% busy) and the shared DMA fabric (~150 GB/s/core
  across 8 cores) under their budgets. z = frep * wt on DVE (fp16 2x).
  The remaining 81 channels (ones-augmented) are read directly from a
  packed [f; w; ones] tensor (chunk "8").
- Contract on the tensor engine in fp16 (fp32 PSUM). Matmuls are issued in
  column-tiled pairs (tile_position (0,0)/(0,64)): two 64-wide PE column
  groups concurrently compute two adjacent pixel tiles into the two
  partition halves of one PSUM bank; pair-slot = N cycles at 2.4 GHz.
- Head choreography: DMA rings are in-order and share one DMA engine +
  the 8-core fabric, so the head runs on two rings (sync+scalar) with
  need-ordered small windows; the gpsimd ring is gated behind a vector
  WAW-memset so it cannot steal head bandwidth. Warmup matmuls (memset-fed)
  hold the PE clock (HAM) at 2.4 GHz until data lands; pair 1's chunk-8
  matmuls are hoisted before pair 0's main chunks to double the PE runway
  while the first z chunks are built.
- Pair 7 covers the tail 7 rows as two N=448 groups (no duplicated row).
- Output layout stays at input width (128): all 9 conv offsets are plain
  column shifts; garbage columns are skipped at DMA-out. PSUM->SBUF
  staging runs on the scalar (Act) engine.
"""

import numpy as np

B, WCH, FCH, OCH, H, W = 4, 16, 64, 64, 128, 128
KX = 3
HO = WO = H - KX + 1          # 126
ROWS_OUT = 63                 # output rows per core
ROWS_IN = 65                  # input rows per core
FREE = 8448                   # padded region width (66 rows * 128)
VALID = ROWS_IN * W           # 8320
NPAIR = 8                     # pixel-tile pairs per core
HALO = 2 * W + 2              # 258
WIN = 1024 + HALO             # 1282: z window per full pair

_cache = {}


def _pair_c0(a):
    return 1024 * a if a < NPAIR - 1 else 7168   # pair 7: rows 56-62, N=448


def _pair_n(a):
    return 512 if a < NPAIR - 1 else 448


def _build_program():
    import concourse.bacc as bacc
    import concourse.mybir as mybir
    import concourse.tile as tile

    f16 = mybir.dt.float16
    f32 = mybir.dt.float32

    nc = bacc.Bacc("TRN2", target_bir_lowering=False)
    fw_d = nc.dram_tensor("fw", (81, FREE), f16, kind="ExternalInput")
    fpre_d = nc.dram_tensor("fpre", (128, FREE), f16, kind="ExternalInput")
    fpw_d = nc.dram_tensor("fpw", (128, NPAIR * 3 * WIN), f16, kind="ExternalInput")
    # extra channels for offsets k=0..2, packed 3x81=243 rows -> 2 full
    # 128-row tiles with the column shift baked in host-side (saves one
    # PE pair-slot per pair vs running those offsets at 81/128 rows).
    fwp2_d = nc.dram_tensor("fwp2", (128, 2 * FREE), f16, kind="ExternalInput")
    wkp2_d = nc.dram_tensor("wkp2", (128, 2 * 64), f16, kind="ExternalInput")
    wt_d = nc.dram_tensor("wt", (128, FREE), f16, kind="ExternalInput")
    wkm_d = nc.dram_tensor("wkm", (128, 9 * 8 * 64), f16, kind="ExternalInput")
    wkx_d = nc.dram_tensor("wkx", (81, 9 * 64), f16, kind="ExternalInput")
    # output stays at input width (128) and fp16: every out-DMA is then a
    # contiguous 64-descriptor transfer; host slices off the 2 pad columns.
    out_d = nc.dram_tensor("out", (OCH, ROWS_OUT * W), f16, kind="ExternalOutput")

    with tile.TileContext(nc) as tc:
        with tc.tile_pool(name="inp", bufs=1) as inp, \
             tc.tile_pool(name="frep", bufs=2) as frp, \
             tc.tile_pool(name="z", bufs=2) as zp, \
             tc.tile_pool(name="st", bufs=3) as stp, \
             tc.tile_pool(name="ps", bufs=4, space="PSUM") as psp:
            # memset-fed warmup matmuls hold the PE clock warm until the
            # first fw window lands (~16us); their PSUM is never read.
            warm = inp.tile([128, 256], f16)
            nc.gpsimd.memset(warm[:], 0.0)
            warm_ps = psp.tile([128, 512], f32, bufs=1)
            for _ in range(58):
                nc.tensor.matmul(warm_ps[0:64, 0:256], warm[:, 0:64], warm[:, 0:256],
                                 start=True, stop=True, tile_position=(0, 0))

            fw_s = inp.tile([81, FREE], f16)
            fpre_s = inp.tile([128, FREE], f16)
            wt_s = inp.tile([128, FREE], f16)
            wkm_s = inp.tile([128, 9 * 8 * 64], f16)
            wkx_s = inp.tile([81, 9 * 64], f16)
            wkp2_s = inp.tile([128, 2 * 64], f16)

            # --- head: two rings, z-chain first, PE inputs after -------
            # (the PE waits warm behind the warmups; the z chain cannot)
            nc.sync.dma_start(fpre_s[:, 0:672], fpre_d[:, 0:672])
            nc.scalar.dma_start(wkx_s[:], wkx_d[:])
            nc.scalar.dma_start(fpre_s[:, 672:1344], fpre_d[:, 672:1344])
            nc.sync.dma_start(wt_s[:, 0:672], wt_d[:, 0:672])
            nc.scalar.dma_start(wt_s[:, 672:1344], wt_d[:, 672:1344])
            nc.sync.dma_start(fw_s[:, 0:1284], fw_d[:, 0:1284])
            nc.scalar.dma_start(fw_s[:, 1284:2368], fw_d[:, 1284:2368])
            nc.sync.dma_start(wkm_s[:, 0:1152], wkm_d[:, 0:1152])
            nc.sync.dma_start(wkp2_s[:], wkp2_d[:])
            nc.scalar.dma_start(wkm_s[:, 1152:], wkm_d[:, 1152:])
            # bulk width, woven into the sync ring in need-order
            sl0 = slice(1344, 3712)
            sl1 = slice(3712, 6080)
            sl2 = slice(6080, FREE)
            fb0 = slice(2368, 4736)
            fb1 = slice(4736, FREE)
            bulk = {
                0: [(fpre_s, fpre_d, sl0), (wt_s, wt_d, sl0)],
                1: [(fw_s, fw_d, fb0), (fpre_s, fpre_d, sl1), (wt_s, wt_d, sl1)],
                2: [(fw_s, fw_d, fb1)],
                3: [(fpre_s, fpre_d, sl2), (wt_s, wt_d, sl2)],
            }

            for a in range(NPAIR):
                c0, n = _pair_c0(a), _pair_n(a)
                win = min(WIN, FREE - c0)
                zz = []
                for c in range(8):
                    fr = frp.tile([128, WIN], f16, tag=f"fc{c}")
                    if c < 5:
                        mask = [2 * c + (r // 16) for r in range(32)]
                        nc.vector.stream_shuffle(fr[:, 0:win], fpre_s[:, c0:c0 + win], mask)
                    else:
                        src = fpw_d[:, (a * 3 + c - 5) * WIN:(a * 3 + c - 4) * WIN]
                        eng = nc.scalar if c == 6 else nc.sync
                        eng.dma_start(fr[:], src)
                    z = zp.tile([128, WIN], f16, tag=f"z{c}")
                    nc.vector.tensor_mul(z[:, 0:win], fr[:, 0:win], wt_s[:, c0:c0 + win])
                    zz.append(z)
                for dst, srcd, sl in bulk.get(a, []):
                    nc.sync.dma_start(dst[:, sl], srcd[:, sl])

                # baked-shift packed extra channels for offsets k=0..2
                fps = []
                for t in (0, 1):
                    fp = frp.tile([128, 2 * 512], f16, tag=f"fp{t}")
                    nc.scalar.dma_start(fp[:, 0:2 * n],
                                        fwp2_d[:, t * FREE + c0:t * FREE + c0 + 2 * n])
                    fps.append(fp)

                ps = psp.tile([128, 512], f32, tag="ps")
                for c in (8, 0, 1, 2, 3, 4, 5, 6, 7):
                    for k in (range(3, 9) if c == 8 else range(9)):
                        dy, dx = divmod(k, KX)
                        d = dy * W + dx
                        for g, off in ((0, 0), (1, n)):
                            if c < 8:
                                lhsT = wkm_s[:, (c * 9 + k) * 64:(c * 9 + k) * 64 + 64]
                                rhs = zz[c][:, d + off:d + off + n]
                            else:
                                lhsT = wkx_s[:, k * 64:k * 64 + 64]
                                rhs = fw_s[:, c0 + d + off:c0 + d + off + n]
                            nc.tensor.matmul(
                                ps[64 * g:64 * g + 64, 0:n], lhsT, rhs,
                                start=(c == 8 and k == 3),
                                stop=False,
                                tile_position=(0, 64 * g),
                            )
                # packed extra slots last: their windows arrive mid-pair
                for t in (0, 1):
                    for g, off in ((0, 0), (1, n)):
                        nc.tensor.matmul(
                            ps[64 * g:64 * g + 64, 0:n],
                            wkp2_s[:, t * 64:t * 64 + 64],
                            fps[t][:, off:off + n],
                            start=False, stop=(t == 1),
                            tile_position=(0, 64 * g),
                        )

                stage = stp.tile([128, 512], f16)
                nc.scalar.copy(stage[:, 0:n], ps[:, 0:n])
                # tail pair's output splits across both rings
                eng1 = nc.sync if a == NPAIR - 1 else nc.scalar
                nc.scalar.dma_start(out_d[:, c0:c0 + n], stage[0:64, 0:n])
                eng1.dma_start(out_d[:, c0 + n:c0 + 2 * n], stage[64:128, 0:n])

    nc.finalize()
    return nc


def _prep_core(inputf, inputw, b, h):
    r0 = 63 * h
    f_reg = np.zeros((64, FREE), np.float16)
    f_reg[:, :VALID] = inputf[b, :, r0:r0 + ROWS_IN, :].reshape(64, VALID)
    w_reg = np.zeros((16, FREE), np.float16)
    w_reg[:, :VALID] = inputw[b, :, r0:r0 + ROWS_IN, :].reshape(16, VALID)
    ones_reg = np.zeros((1, FREE), np.float16)
    ones_reg[0, :VALID] = 1.0
    fw = np.concatenate([f_reg, w_reg, ones_reg], 0)

    # pre-replicated f windows for the DMA-fed chunks 5-7, pair-major
    fpw = np.zeros((128, NPAIR * 3 * WIN), np.float16)
    for c in range(5, 8):
        frep = np.repeat(f_reg[8 * c:8 * c + 8], 16, axis=0)  # [128, FREE]
        for a in range(NPAIR):
            c0 = _pair_c0(a)
            win = min(WIN, FREE - c0)
            fpw[:, (a * 3 + c - 5) * WIN:(a * 3 + c - 5) * WIN + win] = \
                frep[:, c0:c0 + win]

    # shuffle-source layout for chunks 0-3 (quadrant-permuted f rows)
    fpre = np.zeros((128, FREE), np.float16)
    q = np.arange(4)[:, None]
    s = np.arange(16)[None, :]
    rows = (8 * (s // 2) + 2 * q + (s % 2)).reshape(-1)
    idx = (32 * q + s).reshape(-1)
    fpre[idx] = f_reg[rows]

    wt = np.empty((128, FREE), np.float16)
    for u in range(8):
        wt[16 * u:16 * u + 16] = w_reg

    # extra channels for offsets k=0..2 with the column shift baked in:
    # rows g = k*81+e packed into 2 tiles of 128
    fwp2 = np.zeros((2 * 128, FREE), np.float16)
    for k in range(3):
        d = (k // KX) * W + (k % KX)
        fwp2[k * 81:(k + 1) * 81, :FREE - d] = fw[:, d:]
    fwp2 = np.concatenate([fwp2[0:128], fwp2[128:256]], axis=1)  # [128, 2*FREE]
    return fw, fpre, fpw, wt, fwp2


def kernel(inputw, inputf, weight):
    from concourse import bass_utils

    inputw = np.asarray(inputw, np.float32)
    inputf = np.asarray(inputf, np.float32)
    weight = np.asarray(weight, np.float32)

    if "nc" not in _cache:
        _cache["nc"] = _build_program()
    nc = _cache["nc"]

    # weight layouts (replicated across cores)
    p = np.arange(128)
    wkm = np.empty((128, 8, 9, 64), np.float16)
    for t in range(8):
        iw = 8 * t + p // 16
        jw = p % 16
        wkm[:, t, :, :] = weight[:, :, iw, jw].transpose(2, 0, 1)
    wkm = wkm.reshape(128, 8 * 9 * 64)
    wkx = np.empty((81, 9, 64), np.float16)
    wkx[:64] = weight[:, :, :64, 16].transpose(2, 0, 1)
    wkx[64:80] = weight[:, :, 64, :16].transpose(2, 0, 1)
    wkx[80] = weight[:, :, 64, 16]
    wkx = wkx.reshape(81, 9 * 64)
    # packed-extra weights: row g = k*81+e of 2 tiles x 64 out channels
    wkp2 = np.zeros((2 * 128, 64), np.float16)
    for k in range(3):
        wkp2[k * 81:(k + 1) * 81] = wkx.reshape(81, 9, 64)[:, k, :]
    wkp2 = np.concatenate([wkp2[0:128], wkp2[128:256]], axis=1)  # [128, 2*64]

    in_maps = []
    for core in range(8):
        b, h = divmod(core, 2)
        fw, fpre, fpw, wt, fwp2 = _prep_core(inputf, inputw, b, h)
        in_maps.append({"fw": fw, "fpre": fpre, "fpw": fpw, "wt": wt,
                        "wkm": wkm, "wkx": wkx, "fwp2": fwp2, "wkp2": wkp2})

    res = bass_utils.run_bass_kernel_spmd(nc, in_maps, core_ids=list(range(8)))
    kernel.last_result = res

    out = np.empty((B, OCH, HO, WO), np.float32)
    for core in range(8):
        b, h = divmod(core, 2)
        full = res.results[core]["out"].reshape(OCH, ROWS_OUT, W)
        out[b, :, 63 * h:63 * h + 63, :] = full[:, :, 0:WO].astype(np.float32)
    return out
